# Initial kernel scaffold
#
"""Neural ODE (RK4, 2-layer MLP dynamics) Trainium2 Bass kernel.

Strategy: data-parallel over 8 NeuronCores (batch 4096 -> 512/core).
On-chip layout is transposed: hT = [H=256, B=512] stored as one SBUF tile
[128, 1024] (column block k in {0,1} = H-rows [128k, 128k+128)).
The per-core batch is split into 2 halves of 256 columns that pipeline
independently through the engines (breaks the serial RK4 chain).

Per RK4 stage: z = relu(W1 @ inp + b1) (PE matmuls -> ScalarE/VectorE
PSUM eviction with fused bias+relu), k_j via W2 matmuls, u_j = c_j*k_j
evicted with fused scale+bias. tmp = h + u_j adds on GPSIMD/VectorE.
The RK4 combine h' = h + dt/6 (k1+2k2+2k3+k4) is done by accumulating
scaled-identity matmuls ((6/dt)I @ h, (2/dt)I @ u0, ...) into the eval-4
layer-2 PSUM bank, so it costs PE-only work off the critical path.
Per-step output projection W_out @ h -> [64, B] is evicted and DMA'd out;
the host transposes back and adds b_out.
"""

import numpy as np

HIDDEN = 256
OUT = 64
BATCH = 4096
TSTEPS = 100
NCORES = 8
BC = BATCH // NCORES  # 512 batch per core
HB = BC // 2  # 256, half-batch (free dim of most ops)
P = 128

_cache = {}


def _build(dts, dtm):
    """Build the Bass kernel. dts: list of 99 python-float step sizes,
    dtm: mean dt (used for the identity-injection matrices and the final
    combine scale so the h coefficient is exactly 1)."""
    import concourse.bass as bass
    import concourse.mybir as mybir
    from contextlib import ExitStack
    from concourse.tile import TileContext

    f32 = mybir.dt.float32
    AF = mybir.ActivationFunctionType
    ALU = mybir.AluOpType

    nc = bass.Bass()

    xT = nc.dram_tensor("xT", [OUT, BC], f32, kind="ExternalInput")
    winT_d = nc.dram_tensor("winT", [OUT, HIDDEN], f32, kind="ExternalInput")
    w1T_d = nc.dram_tensor("w1T", [P, 512], f32, kind="ExternalInput")
    w2T_d = nc.dram_tensor("w2T", [P, 512], f32, kind="ExternalInput")
    woutT_d = nc.dram_tensor("woutT", [P, 128], f32, kind="ExternalInput")
    ident_d = nc.dram_tensor("ident", [P, 384], f32, kind="ExternalInput")
    bias_d = nc.dram_tensor("biases", [P, 10], f32, kind="ExternalInput")
    out_d = nc.dram_tensor("out", [TSTEPS, OUT, BC], f32, kind="ExternalOutput")

    nsteps = len(dts)  # 99

    with TileContext(nc) as tc, ExitStack() as ctx:
        const = ctx.enter_context(tc.tile_pool(name="const", bufs=1))
        hpool = ctx.enter_context(tc.tile_pool(name="hpool", bufs=2))
        zpool = ctx.enter_context(tc.tile_pool(name="zpool", bufs=4))
        upool = ctx.enter_context(tc.tile_pool(name="upool", bufs=2))
        tpool = ctx.enter_context(tc.tile_pool(name="tpool", bufs=4))
        opool = ctx.enter_context(tc.tile_pool(name="opool", bufs=4))
        pa = ctx.enter_context(tc.tile_pool(name="pa", bufs=3, space="PSUM"))
        pb = ctx.enter_context(tc.tile_pool(name="pb", bufs=3, space="PSUM"))
        po = ctx.enter_context(tc.tile_pool(name="po", bufs=2, space="PSUM"))

        # ---- load constants into SBUF
        x_sb = const.tile([OUT, BC], f32, name="x_sb")
        win = const.tile([OUT, HIDDEN], f32, name="win")
        w1 = const.tile([P, 512], f32, name="w1")
        w2 = const.tile([P, 512], f32, name="w2")
        wout = const.tile([P, 128], f32, name="wout")
        ident = const.tile([P, 384], f32, name="ident")
        bia = const.tile([P, 10], f32, name="bia")
        nc.sync.dma_start(x_sb[:], xT[:, :])
        nc.sync.dma_start(win[:], winT_d[:, :])
        nc.sync.dma_start(w1[:], w1T_d[:, :])
        nc.sync.dma_start(w2[:], w2T_d[:, :])
        nc.sync.dma_start(wout[:], woutT_d[:, :])
        nc.sync.dma_start(ident[:], ident_d[:, :])
        nc.sync.dma_start(bia[:], bias_d[:, :])

        # identity blocks: I2=(2/dtm)I, I4=(4/dtm)I, I6=(6/dtm)I
        I2 = ident[:, 0:128]
        I4 = ident[:, 128:256]
        I6 = ident[:, 256:384]

        def bcol(j):  # [128,1] bias column
            return bia[:, j : j + 1]

        # bias columns: 0,1 b_in(m); 2,3 b1(m); 4,5 (dtm/2)b2; 6,7 dtm*b2; 8,9 (dtm/6)b2

        # ---- h0 = W_in @ xT + b_in   (full batch, N=512)
        h = hpool.tile([P, 1024], f32, tag="h", name="h0")
        for m in range(2):
            ps = pa.tile([P, BC], f32, tag="pinit", name="ps_init")
            nc.tensor.matmul(
                ps[:], win[:, m * 128 : (m + 1) * 128], x_sb[:], start=True, stop=True
            )
            nc.scalar.activation(
                h[:, m * 512 : (m + 1) * 512], ps[:], AF.Identity, bias=bcol(m), scale=1.0
            )

        # W1T block (k, m) / W2T block (k, m) column ranges
        def wblk(w, k, m):
            j = (k * 2 + m) * 128
            return w[:, j : j + 128]

        def emit_outproj(t, h, b):
            pso = po.tile([OUT, HB], f32, tag="pso", name="pso")
            for k in range(2):
                nc.tensor.matmul(
                    pso[:],
                    wout[:, k * 64 : (k + 1) * 64],
                    h[:, k * 512 + b * HB : k * 512 + b * HB + HB],
                    start=(k == 0),
                    stop=(k == 1),
                )
            osb = opool.tile([OUT, HB], f32, tag="osb", name="osb")
            if b == 0:
                nc.scalar.copy(osb[:], pso[:])
            else:
                nc.vector.tensor_copy(osb[:], pso[:])
            nc.sync.dma_start(out_d[t, :, b * HB : (b + 1) * HB], osb[:])

        # ---- time stepping
        for t in range(nsteps):
            dt = dts[t]
            hn = hpool.tile([P, 1024], f32, tag="h", name="hn")

            # per-half state
            inp = [h, h]  # current eval input (full-layout tile w/ views)
            inp_is_h = [True, True]
            pb4 = [None, None]
            us = [[None] * 3, [None] * 3]

            # eval-4 layer-2 psum: open accumulation group early with
            # (6/dtm) I @ h so injections run off the critical path.
            for b in range(2):
                pb4[b] = pb.tile([P, HB, 2], f32, tag="pb4", name="pb4")
                # note: allocate as [P, HB, 2]?? -- no, see below; kept 2D
            # (re-do: separate tiles per m)
            pb4 = [[None, None], [None, None]]
            for b in range(2):
                for m in range(2):
                    pt = pb.tile([P, HB], f32, tag=f"pb4_{m}", name="pb4")
                    pb4[b][m] = pt
                    nc.tensor.matmul(
                        pt[:],
                        I6,
                        h[:, m * 512 + b * HB : m * 512 + b * HB + HB],
                        start=True,
                        stop=False,
                    )

            for b in range(2):
                emit_outproj(t, h, b)

            for j in range(4):  # RK4 stages
                for b in range(2):
                    # input views for this stage
                    if inp_is_h[b]:
                        iv = [
                            h[:, k * 512 + b * HB : k * 512 + b * HB + HB]
                            for k in range(2)
                        ]
                    else:
                        tmp = inp[b]
                        iv = [tmp[:, k * HB : (k + 1) * HB] for k in range(2)]

                    # layer 1: psumA[m] = sum_k W1T(k,m) @ iv[k]
                    pA = []
                    for m in range(2):
                        pt = pa.tile([P, HB], f32, tag="pA", name="pA")
                        pA.append(pt)
                    for k in range(2):
                        for m in range(2):
                            nc.tensor.matmul(
                                pA[m][:], wblk(w1, k, m), iv[k],
                                start=(k == 0), stop=(k == 1),
                            )
                    # relu eviction (bias b1), split ACT / DVE
                    z = zpool.tile([P, 2 * HB], f32, tag="z", name="z")
                    nc.scalar.activation(
                        z[:, 0:HB], pA[0][:], AF.Relu, bias=bcol(2), scale=1.0
                    )
                    nc.vector.tensor_scalar(
                        z[:, HB : 2 * HB], pA[1][:], bcol(3), 0.0,
                        op0=ALU.add, op1=ALU.max,
                    )

                    # layer 2: psumB[m] = sum_k W2T(k,m) @ z[k]
                    if j < 3:
                        pB = []
                        for m in range(2):
                            pt = pb.tile([P, HB], f32, tag="pB", name="pB")
                            pB.append(pt)
                        for k in range(2):
                            for m in range(2):
                                nc.tensor.matmul(
                                    pB[m][:], wblk(w2, k, m), z[:, k * HB : (k + 1) * HB],
                                    start=(k == 0), stop=(k == 1),
                                )
                        # u_j = c_j * k_j eviction with fused scale+bias
                        c = (dt / 2.0, dt / 2.0, dt)[j]
                        bc0 = (4, 4, 6)[j]  # bias col pair base
                        u = upool.tile([P, 2 * HB], f32, tag=f"u{j}", name="u")
                        us[b][j] = u
                        nc.scalar.activation(
                            u[:, 0:HB], pB[0][:], AF.Identity, bias=bcol(bc0), scale=c
                        )
                        nc.vector.tensor_scalar(
                            u[:, HB : 2 * HB], pB[1][:], c, bcol(bc0 + 1),
                            op0=ALU.mult, op1=ALU.add,
                        )
                        # inject (cI) @ u_j into pb4 accumulation group
                        Ij = (I2, I4, I2)[j]
                        for m in range(2):
                            nc.tensor.matmul(
                                pb4[b][m][:], Ij, u[:, m * HB : (m + 1) * HB],
                                start=False, stop=False,
                            )
                        # tmp = h + u_j  (input of next stage)
                        tmp = tpool.tile([P, 2 * HB], f32, tag="tmp", name="tmp")
                        nc.gpsimd.tensor_tensor(
                            tmp[:, 0:HB],
                            h[:, b * HB : b * HB + HB],
                            u[:, 0:HB],
                            op=ALU.add,
                        )
                        nc.vector.tensor_tensor(
                            tmp[:, HB : 2 * HB],
                            h[:, 512 + b * HB : 512 + b * HB + HB],
                            u[:, HB : 2 * HB],
                            op=ALU.add,
                        )
                        inp[b] = tmp
                        inp_is_h[b] = False
                    else:
                        # final stage: W2 @ z4 into the pb4 group, then evict h_new
                        for k in range(2):
                            for m in range(2):
                                nc.tensor.matmul(
                                    pb4[b][m][:], wblk(w2, k, m),
                                    z[:, k * HB : (k + 1) * HB],
                                    start=False, stop=(k == 1),
                                )
                        sc = dtm / 6.0
                        nc.scalar.activation(
                            hn[:, b * HB : b * HB + HB], pb4[b][0][:],
                            AF.Identity, bias=bcol(8), scale=sc,
                        )
                        nc.vector.tensor_scalar(
                            hn[:, 512 + b * HB : 512 + b * HB + HB], pb4[b][1][:],
                            sc, bcol(9), op0=ALU.mult, op1=ALU.add,
                        )
            h = hn

        # final output projection (t = nsteps)
        for b in range(2):
            emit_outproj(nsteps, h, b)

    return nc


def _prep_shared(W_in, b_in, W1, b1, W2, b2, W_out, dtm):
    f = np.float32

    def pack_blocks(WT):  # [256,256] -> [128, 512] blocks (k*2+m)
        blks = [
            WT[k * 128 : (k + 1) * 128, m * 128 : (m + 1) * 128]
            for k in range(2)
            for m in range(2)
        ]
        return np.ascontiguousarray(np.concatenate(blks, axis=1), dtype=f)

    winT = np.ascontiguousarray(W_in.T, dtype=f)  # [64, 256]
    w1T = pack_blocks(W1.T.astype(f))
    w2T = pack_blocks(W2.T.astype(f))
    woutT = np.ascontiguousarray(W_out.T, dtype=f)  # [256, 64] -> [128,128] blocks
    woutT = np.concatenate([woutT[0:128, :], woutT[128:256, :]], axis=1)
    woutT = np.ascontiguousarray(woutT, dtype=f)

    I = np.eye(128, dtype=f)
    ident = np.concatenate(
        [(f(2.0) / dtm) * I, (f(4.0) / dtm) * I, (f(6.0) / dtm) * I], axis=1
    ).astype(f)

    def cols2(v):  # [256] -> two [128] cols
        return [v[0:128], v[128:256]]

    b2 = b2.astype(f)
    cols = (
        cols2(b_in.astype(f))
        + cols2(b1.astype(f))
        + cols2((dtm / f(2.0)) * b2)
        + cols2(dtm * b2)
        + cols2((dtm / f(6.0)) * b2)
    )
    biases = np.stack(cols, axis=1).astype(f)  # [128, 10]
    return dict(winT=winT, w1T=w1T, w2T=w2T, woutT=woutT, ident=ident, biases=biases)


_last_results = None


def kernel(x, t_span, W_in, b_in, W1, b1, W2, b2, W_out, b_out):
    global _last_results
    from concourse.bass_utils import run_bass_kernel_spmd

    f = np.float32
    x = np.asarray(x, f)
    t_span = np.asarray(t_span, f)
    dts = np.diff(t_span).astype(f)
    dtm = f(dts.mean())

    key = dts.tobytes()
    if key not in _cache:
        _cache[key] = _build([float(d) for d in dts], float(dtm))
    nc = _cache[key]

    shared = _prep_shared(
        np.asarray(W_in), np.asarray(b_in), np.asarray(W1), np.asarray(b1),
        np.asarray(W2), np.asarray(b2), np.asarray(W_out), dtm,
    )
    in_maps = []
    for c in range(NCORES):
        xc = np.ascontiguousarray(x[c * BC : (c + 1) * BC].T, dtype=f)  # [64, 512]
        m = dict(shared)
        m["xT"] = xc
        in_maps.append(m)

    res = run_bass_kernel_spmd(nc, in_maps, core_ids=list(range(NCORES)))
    _last_results = res
    outs = [r["out"] for r in res.results]  # each [100, 64, 512]
    full = np.concatenate([o.transpose(0, 2, 1) for o in outs], axis=1)
    full = full + np.asarray(b_out, f)[None, None, :]
    return np.ascontiguousarray(full, dtype=f)


# revision 21
# speedup vs baseline: 1.0038x; 1.0038x over previous
"""Neural ODE (RK4, 2-layer MLP dynamics) Trainium2 Bass kernel.

Strategy: data-parallel over 8 NeuronCores (batch 4096 -> 512/core).
On-chip layout is transposed: hT = [H=256, B=512] stored as one SBUF tile
[128, 1024] (column block k in {0,1} = H-rows [128k, 128k+128)).
The per-core batch is split into 2 halves of 256 columns that pipeline
independently through the engines (breaks the serial RK4 chain).

Matmul operands are float32r (relaxed-precision fp32, same bytes): the PE
streams f32r at 1 cycle/row vs 4 for strict fp32. The integration state h
is kept in strict fp32 and updated only by a VectorE/GPSIMD add
(h' = h + s), so state precision does not degrade across the 99 steps;
a rounded copy h_r feeds the matmuls.

Per RK4 stage: z = relu(W1 @ inp + b1) (PE matmuls, per-m PSUM banks ->
ScalarE(m0)/VectorE(m1) evictions with fused bias+relu), k_j via W2
matmuls, u_j = c_j*k_j evicted with fused scale+bias. tmp = h + u_j adds
on GPSIMD/VectorE. The RK4 increment s = u0/3 + 2u1/3 + u2/3 + dt/6*k4
is accumulated in a PSUM bank via scaled-identity matmuls (off the
critical path); h' = h + s runs at full fp32.
Per-step output projection W_out @ h -> [64, B] is evicted and DMA'd out;
the host transposes back and adds b_out.

PSUM note: matmul start=True clears the has_written bits of the ENTIRE
bank, so exactly one matmul per bank incarnation carries it; start=False
matmuls overwrite fresh regions (has_written=0) and accumulate written
ones. Banks: pA m0/m1 (2) + pB m0/m1 (2+2) + shared pso/pb4 pool (2) = 8.
"""

import numpy as np

HIDDEN = 256
OUT = 64
BATCH = 4096
TSTEPS = 100
NCORES = 8
BC = BATCH // NCORES  # 512 batch per core
HB = BC // 2  # 256, half-batch (free dim of most ops)
P = 128

_cache = {}


ENG = {  # engine assignment knobs (sim-tuned)
    "relu_m1": "dve", "u_m1": "dve", "s_m1": "dve",
    "tmp_k0": "dve", "tmp_k1": "gps",
    "hn_k0": "dve", "hn_k1": "gps",
    "hr_k0": "dve", "hr_k1": "act",
    "osb_b1": "dve",
}


def _build(dts, dtm, debug_dump=False, eng=None):
    """Build the Bass kernel. dts: 99 python-float step sizes, dtm: mean dt
    (used for the identity-injection matrices and the combine scale so the
    u_j combine coefficients are exact)."""
    import concourse.bass as bass
    import concourse.mybir as mybir
    from contextlib import ExitStack
    from concourse.bacc import Bacc
    from concourse.tile import TileContext

    f32 = mybir.dt.float32
    f32r = mybir.dt.float32r
    AF = mybir.ActivationFunctionType
    ALU = mybir.AluOpType

    E = dict(ENG)
    if eng:
        E.update(eng)

    nc = Bacc("TRN2", target_bir_lowering=False, debug=False)

    xT = nc.dram_tensor("xT", [OUT, BC], f32r, kind="ExternalInput")
    winT_d = nc.dram_tensor("winT", [OUT, HIDDEN], f32r, kind="ExternalInput")
    w1T_d = nc.dram_tensor("w1T", [P, 512], f32r, kind="ExternalInput")
    w2T_d = nc.dram_tensor("w2T", [P, 512], f32r, kind="ExternalInput")
    woutT_d = nc.dram_tensor("woutT", [P, 128], f32r, kind="ExternalInput")
    ident_d = nc.dram_tensor("ident", [P, 384], f32r, kind="ExternalInput")
    bias_d = nc.dram_tensor("biases", [P, 10], f32, kind="ExternalInput")
    out_d = nc.dram_tensor("out", [TSTEPS, OUT, BC], f32, kind="ExternalOutput")

    nsteps = len(dts)  # 99
    dbg = {}
    if debug_dump:
        for nm in ("z1d", "u0d", "u1d", "u2d", "z4d", "h1d"):
            dbg[nm] = nc.dram_tensor(nm, [P, 1024], f32, kind="ExternalOutput")

    with TileContext(nc) as tc, ExitStack() as ctx:
        const = ctx.enter_context(tc.tile_pool(name="const", bufs=1))
        hpool = ctx.enter_context(tc.tile_pool(name="hpool", bufs=2))
        hrpool = ctx.enter_context(tc.tile_pool(name="hrpool", bufs=2))
        zpool = ctx.enter_context(tc.tile_pool(name="zpool", bufs=4))
        upool = ctx.enter_context(tc.tile_pool(name="upool", bufs=2))
        tpool = ctx.enter_context(tc.tile_pool(name="tpool", bufs=4))
        spool = ctx.enter_context(tc.tile_pool(name="spool", bufs=2))
        opool = ctx.enter_context(tc.tile_pool(name="opool", bufs=4))
        # PSUM banks: pA0/pA1 (1+1) + pB0/pB1 (2+2) + pso/pb4 shared (2) = 8
        pa = ctx.enter_context(
            tc.tile_pool(name="pa", bufs=int(E.get("pa_bufs", 1)), space="PSUM")
        )
        pbp = ctx.enter_context(
            tc.tile_pool(name="pbp", bufs=int(E.get("pb_bufs", 2)), space="PSUM")
        )
        p4p = ctx.enter_context(tc.tile_pool(name="p4p", bufs=2, space="PSUM"))

        # ---- load constants into SBUF
        x_sb = const.tile([OUT, BC], f32r, name="x_sb")
        win = const.tile([OUT, HIDDEN], f32r, name="win")
        w1 = const.tile([P, 512], f32r, name="w1")
        w2 = const.tile([P, 512], f32r, name="w2")
        wout = const.tile([P, 128], f32r, name="wout")
        ident = const.tile([P, 384], f32r, name="ident")
        bia = const.tile([P, 10], f32, name="bia")
        nc.sync.dma_start(x_sb[:], xT[:, :])
        nc.sync.dma_start(win[:], winT_d[:, :])
        nc.sync.dma_start(w1[:], w1T_d[:, :])
        nc.sync.dma_start(w2[:], w2T_d[:, :])
        nc.sync.dma_start(wout[:], woutT_d[:, :])
        nc.sync.dma_start(ident[:], ident_d[:, :])
        nc.sync.dma_start(bia[:], bias_d[:, :])

        # PE matmuls may carry at most ONE sync wait; absorb every const-DMA
        # queue tick into the PE vector clock up front via dummy 1x1 matmuls.
        dmy = p4p.tile([1, 1], f32, tag="p4", name="dmy")
        for cst in (x_sb, win, w1, w2, wout, ident, bia):
            c1 = cst[:, 0:1].bitcast(f32)  # f32r 1x1 matmul is invalid ISA
            nc.tensor.matmul(
                dmy[:], c1, c1, start=True, stop=True, skip_group_check=True
            )

        I2 = ident[:, 0:128]  # (2/dtm) I
        I4 = ident[:, 128:256]  # (4/dtm) I

        def bcol(j):  # [128,1] bias column
            return bia[:, j : j + 1]

        # bias cols: 0,1 b_in(m); 2,3 b1(m); 4,5 (dtm/2)b2; 6,7 dtm*b2; 8,9 (dtm/6)b2

        def wblk(w, k, m):  # W1T/W2T block (k, m)
            j = (k * 2 + m) * 128
            return w[:, j : j + 128]

        def new_h(b):
            return hpool.tile([P, 2 * HB], f32, tag=f"hb{b}", name="h")

        def new_hr(b):
            return hrpool.tile([P, 2 * HB], f32r, tag=f"hrb{b}", name="hr")

        def kv(hh_b, k):  # k-chunk view of a per-half tile
            return hh_b[:, k * HB : (k + 1) * HB]

        # ---- h0 = W_in @ xT + b_in   (full batch, N=512)
        h = [new_h(0), new_h(1)]
        hr = [new_hr(0), new_hr(1)]
        for m in range(2):
            ps = pa.tile([P, BC], f32, tag=f"pA{m}", name="ps_init")
            nc.tensor.matmul(
                ps[:], win[:, m * 128 : (m + 1) * 128], x_sb[:], start=True, stop=True
            )
            for b in range(2):
                src = ps[:, b * HB : (b + 1) * HB]
                if b == 0:
                    nc.scalar.activation(
                        kv(h[b], m), src, AF.Identity, bias=bcol(m), scale=1.0
                    )
                else:
                    nc.vector.tensor_scalar(
                        kv(h[b], m), src, bcol(m), None, op0=ALU.add
                    )
        for b in range(2):
            nc.vector.tensor_copy(hr[b][:], h[b][:])

        def emit_outproj(t, hr_b, b):
            pso = p4p.tile([OUT, HB], f32, tag="p4", name="pso")
            for k in range(2):
                nc.tensor.matmul(
                    pso[:], wout[:, k * 64 : (k + 1) * 64], kv(hr_b, k),
                    start=(k == 0), stop=(k == 1),
                )
            osb = opool.tile([OUT, HB], f32, tag=f"osb{b}", name="osb")
            if b == 0 or E["osb_b1"] == "act":
                nc.scalar.copy(osb[:], pso[:])
            else:
                nc.vector.tensor_copy(osb[:], pso[:])
            nc.sync.dma_start(out_d[t, :, b * HB : (b + 1) * HB], osb[:])

        # ---- time stepping
        for t in range(nsteps):
            dt = dts[t]
            hn = [new_h(0), new_h(1)]
            hrn = [new_hr(0), new_hr(1)]

            inp = [None, None]  # [b] -> list of per-k input views
            pb4 = [None, None]

            for b in range(2):
                emit_outproj(t, hr[b], b)

            for j in range(4):  # RK4 stages
                for b in range(2):
                    if inp[b] is None:
                        iv = [kv(hr[b], 0), kv(hr[b], 1)]
                    else:
                        iv = inp[b]

                    # layer 1: per-m banks, m0 group first so its eviction
                    # starts while m1's matmuls run
                    pA = []
                    for m in range(2):
                        pt = pa.tile([P, HB], f32, tag=f"pA{m}", name="pAt")
                        pA.append(pt)
                        for k in range(2):
                            nc.tensor.matmul(
                                pt[:], wblk(w1, k, m), iv[k],
                                start=(k == 0), stop=(k == 1),
                                skip_group_check=True,
                            )
                    z = [
                        zpool.tile([P, HB], f32r, tag="z0", name="z0"),
                        zpool.tile([P, HB], f32r, tag="z1", name="z1"),
                    ]
                    nc.scalar.activation(
                        z[0][:], pA[0][:], AF.Relu, bias=bcol(2), scale=1.0
                    )
                    if E["relu_m1"] == "dve":
                        nc.vector.tensor_scalar(
                            z[1][:], pA[1][:], bcol(3), 0.0, op0=ALU.add, op1=ALU.max
                        )
                    else:
                        nc.scalar.activation(
                            z[1][:], pA[1][:], AF.Relu, bias=bcol(3), scale=1.0
                        )

                    if debug_dump and t == 0 and j == 0:
                        for m in range(2):
                            nc.sync.dma_start(
                                dbg["z1d"][:, b * 512 + m * HB : b * 512 + (m + 1) * HB],
                                z[m][:],
                            )
                    if debug_dump and t == 0 and j == 3:
                        for m in range(2):
                            nc.sync.dma_start(
                                dbg["z4d"][:, b * 512 + m * HB : b * 512 + (m + 1) * HB],
                                z[m][:],
                            )

                    if j < 3:
                        # layer 2: per-m banks
                        pB = []
                        for m in range(2):
                            pt = pbp.tile([P, HB], f32, tag=f"pB{m}", name="pBt")
                            pB.append(pt)
                            for k in range(2):
                                nc.tensor.matmul(
                                    pt[:], wblk(w2, k, m), z[k][:],
                                    start=(k == 0), stop=(k == 1),
                                    skip_group_check=True,
                                )
                        # u_j = c_j * k_j eviction with fused scale+bias
                        c = (dt / 2.0, dt / 2.0, dt)[j]
                        bc0 = (4, 4, 6)[j]
                        u = [
                            upool.tile([P, HB], f32r, tag=f"u{j}m0", name="u0"),
                            upool.tile([P, HB], f32r, tag=f"u{j}m1", name="u1"),
                        ]
                        nc.scalar.activation(
                            u[0][:], pB[0][:], AF.Identity, bias=bcol(bc0), scale=c
                        )
                        if E["u_m1"] == "dve":
                            nc.vector.tensor_scalar(
                                u[1][:], pB[1][:], c, bcol(bc0 + 1),
                                op0=ALU.mult, op1=ALU.add,
                            )
                        else:
                            nc.scalar.activation(
                                u[1][:], pB[1][:], AF.Identity, bias=bcol(bc0 + 1),
                                scale=c,
                            )
                        if debug_dump and t == 0:
                            for m in range(2):
                                nc.sync.dma_start(
                                    dbg[f"u{j}d"][
                                        :, b * 512 + m * HB : b * 512 + (m + 1) * HB
                                    ],
                                    u[m][:],
                                )
                        # inject (cI) @ u_j into the pb4 increment accumulator
                        if j == 0:
                            pb4[b] = p4p.tile([P, 2 * HB], f32, tag="p4", name="pb4")
                        Ij = (I2, I4, I2)[j]
                        for m in range(2):
                            nc.tensor.matmul(
                                pb4[b][:, m * HB : (m + 1) * HB], Ij, u[m][:],
                                start=(j == 0 and m == 0), stop=False,
                                skip_group_check=True,
                            )
                        # tmp = h + u_j  (h read at full fp32, written rounded
                        # to f32r for the matmuls)
                        tmp = [
                            tpool.tile([P, HB], f32r, tag="tmp0", name="t0"),
                            tpool.tile([P, HB], f32r, tag="tmp1", name="t1"),
                        ]
                        tte = {"dve": nc.vector, "gps": nc.gpsimd}
                        tte[E["tmp_k0"]].tensor_tensor(
                            tmp[0][:], kv(h[b], 0).bitcast(f32r), u[0][:], op=ALU.add
                        )
                        tte[E["tmp_k1"]].tensor_tensor(
                            tmp[1][:], kv(h[b], 1).bitcast(f32r), u[1][:], op=ALU.add
                        )
                        inp[b] = [tmp[0][:], tmp[1][:]]
                    else:
                        # final stage: W2 @ z4 into pb4 (m0 chunk fully first)
                        for m in range(2):
                            for k in range(2):
                                nc.tensor.matmul(
                                    pb4[b][:, m * HB : (m + 1) * HB], wblk(w2, k, m),
                                    z[k][:],
                                    start=False, stop=(k == 1 and m == 1),
                                    skip_group_check=True,
                                )
                        # increment s = u0/3 + 2u1/3 + u2/3 + (dtm/6)(W2@z4+b2)
                        sc = dtm / 6.0
                        sl = [
                            spool.tile([P, HB], f32, tag="s0", name="s0"),
                            spool.tile([P, HB], f32, tag="s1", name="s1"),
                        ]
                        nc.scalar.activation(
                            sl[0][:], pb4[b][:, 0:HB], AF.Identity,
                            bias=bcol(8), scale=sc,
                        )
                        if E["s_m1"] == "dve":
                            nc.vector.tensor_scalar(
                                sl[1][:], pb4[b][:, HB : 2 * HB], sc, bcol(9),
                                op0=ALU.mult, op1=ALU.add,
                            )
                        else:
                            nc.scalar.activation(
                                sl[1][:], pb4[b][:, HB : 2 * HB], AF.Identity,
                                bias=bcol(9), scale=sc,
                            )
                        # full-precision state update h' = h + s (fp32)
                        tte = {"dve": nc.vector, "gps": nc.gpsimd}
                        tte[E["hn_k0"]].tensor_tensor(
                            kv(hn[b], 0), kv(h[b], 0), sl[0][:], op=ALU.add
                        )
                        tte[E["hn_k1"]].tensor_tensor(
                            kv(hn[b], 1), kv(h[b], 1), sl[1][:], op=ALU.add
                        )

                        # rounded copy for next step's matmuls
                        def _copy(engn, dst, src):
                            if engn == "dve":
                                nc.vector.tensor_copy(dst, src)
                            else:
                                nc.scalar.copy(dst, src)

                        _copy(E["hr_k0"], kv(hrn[b], 0), kv(hn[b], 0))
                        _copy(E["hr_k1"], kv(hrn[b], 1), kv(hn[b], 1))
            if debug_dump and t == 0:
                for b in range(2):
                    for k in range(2):
                        nc.sync.dma_start(
                            dbg["h1d"][:, k * 512 + b * HB : k * 512 + (b + 1) * HB],
                            kv(hn[b], k),
                        )
            h = hn
            hr = hrn

        # final output projection (t = nsteps)
        for b in range(2):
            emit_outproj(nsteps, hr[b], b)

    nc.compile()  # bacc passes: event-sem legalization, reg alloc, DCE
    return nc


def _prep_shared(W_in, b_in, W1, b1, W2, b2, W_out, dtm):
    f = np.float32

    def pack_blocks(WT):  # [256,256] -> [128, 512] blocks (k*2+m)
        blks = [
            WT[k * 128 : (k + 1) * 128, m * 128 : (m + 1) * 128]
            for k in range(2)
            for m in range(2)
        ]
        return np.ascontiguousarray(np.concatenate(blks, axis=1), dtype=f)

    winT = np.ascontiguousarray(W_in.T, dtype=f)  # [64, 256]
    w1T = pack_blocks(W1.T.astype(f))
    w2T = pack_blocks(W2.T.astype(f))
    wt = W_out.T.astype(f)  # [256, 64]
    woutT = np.ascontiguousarray(
        np.concatenate([wt[0:128, :], wt[128:256, :]], axis=1), dtype=f
    )  # [128, 128]

    I = np.eye(128, dtype=f)
    ident = np.ascontiguousarray(
        np.concatenate(
            [(f(2.0) / dtm) * I, (f(4.0) / dtm) * I, (f(6.0) / dtm) * I], axis=1
        ),
        dtype=f,
    )

    def cols2(v):  # [256] -> two [128] cols
        return [v[0:128], v[128:256]]

    b2 = b2.astype(f)
    cols = (
        cols2(b_in.astype(f))
        + cols2(b1.astype(f))
        + cols2((dtm / f(2.0)) * b2)
        + cols2(dtm * b2)
        + cols2((dtm / f(6.0)) * b2)
    )
    biases = np.ascontiguousarray(np.stack(cols, axis=1), dtype=f)  # [128, 10]
    return dict(winT=winT, w1T=w1T, w2T=w2T, woutT=woutT, ident=ident, biases=biases)


_last_results = None


def kernel(x, t_span, W_in, b_in, W1, b1, W2, b2, W_out, b_out):
    global _last_results
    from concourse.bass_utils import run_bass_kernel_spmd

    f = np.float32
    x = np.asarray(x, f)
    t_span = np.asarray(t_span, f)
    dts = np.diff(t_span).astype(f)
    dtm = f(dts.mean())

    key = dts.tobytes()
    if key not in _cache:
        _cache[key] = _build([float(d) for d in dts], float(dtm))
    nc = _cache[key]

    shared = _prep_shared(
        np.asarray(W_in), np.asarray(b_in), np.asarray(W1), np.asarray(b1),
        np.asarray(W2), np.asarray(b2), np.asarray(W_out), dtm,
    )
    in_maps = []
    for c in range(NCORES):
        xc = np.ascontiguousarray(x[c * BC : (c + 1) * BC].T, dtype=f)  # [64, 512]
        m = dict(shared)
        m["xT"] = xc
        in_maps.append(m)

    res = run_bass_kernel_spmd(nc, in_maps, core_ids=list(range(NCORES)))
    _last_results = res
    outs = [np.asarray(r["out"]) for r in res.results]  # each [100, 64, 512]
    full = np.concatenate([o.transpose(0, 2, 1) for o in outs], axis=1)
    full = full + np.asarray(b_out, f)[None, None, :]
    return np.ascontiguousarray(full, dtype=f)
